# revision 5
# baseline (speedup 1.0000x reference)
"""BottomUpGNN Trainium2 kernel: 8-core graph-data-parallel message passing.

Sharding: nodes of each type split contiguously over 8 cores; within each
core/type, message-receiving nodes are permuted so nodes active at depth step
s form contiguous 512-aligned groups (each node updates at most once, at its
own depth).  A node-major replica R of post-hidden-MLP states is built once
via AllGather; per-step updates are AllGathered into compact arrays U_s.
Edge aggregation = dma_gather of source rows + one-hot matmuls accumulating
agg[feat, dstslot] in PSUM (no scatter DMA).  MLPs run feature-major with
fused GELU+bias on ScalarE.

Node-major arrays (R, U_s) store a region of n rows in a "wrapped" order:
padded-position q lives at row (q % 128) * (n // 128) + q // 128, so that the
transpose-and-stage DMAs write per-partition-contiguous chunks.  Gather
indices are precomputed on the host against the wrapped layout.
"""

import os
from collections import Counter, defaultdict

import numpy as np

MAX_DEPTH = 4
REL_PER_GRAPH = 32
NCORES = 8
NODE_COUNTS = {"Table": 8192, "Field": 32768, "OP": 65536, "Rel": 32768, "Literal": 32768}
INPUT_DIMS = {"Table": 64, "Field": 96, "OP": 128, "Rel": 160, "Literal": 32}
HIDDEN_DIM, OUT_DIM = 256, 128
EDGE_TYPES = [("Rel", "Rel"), ("Rel", "OP"), ("OP", "Rel"), ("OP", "OP"),
              ("Field", "OP"), ("Field", "Rel"), ("Table", "Field"), ("Literal", "OP")]
DST_TYPES = ("Rel", "OP", "Field")
ALL_TYPES = ("Table", "Field", "OP", "Rel", "Literal")
R_ORDER = ("Table", "Literal", "Field", "OP", "Rel")
WIN = 32768          # rows addressable by one dma_gather call (int16)
SBW = 512            # dst-slot superblock width (one PSUM bank at fp32)
TILE = 128           # edges per aggregation matmul tile
MAX_GCALL = 4096     # max idxs per dma_gather call
NCHUNK = 256         # node chunk width for MLP matmuls
SGRP = 8             # transpose chunks (of 128) batched per staging DMA
DEBUG = os.environ.get("GNN_DEBUG", "") == "1"


def _ceil(a, b):
    return (a + b - 1) // b


def _pad_to(x, m):
    return _ceil(x, m) * m


# ---------------------------------------------------------------------------
# host-side planning
# ---------------------------------------------------------------------------

def make_plan(inputs):
    depths = {t: np.asarray(inputs["depth_" + t.lower()]) for t in ALL_TYPES}
    edges = {st_dt: np.asarray(inputs["edge_%s_%s" % (st_dt[0].lower(), st_dt[1].lower())])
             for st_dt in EDGE_TYPES}
    nloc = {t: NODE_COUNTS[t] // NCORES for t in ALL_TYPES}

    indeg = {t: np.zeros(NODE_COUNTS[t], np.int64) for t in DST_TYPES}
    for (st, dt), e in edges.items():
        np.add.at(indeg[dt], e[1], 1)
    nstep = {t: np.where(indeg[t] > 0, depths[t], 0).astype(np.int64) for t in DST_TYPES}

    G = {t: np.zeros(MAX_DEPTH + 1, np.int64) for t in DST_TYPES}
    counts = {t: np.zeros((NCORES, MAX_DEPTH + 1), np.int64) for t in DST_TYPES}
    for t in DST_TYPES:
        for c in range(NCORES):
            sl = nstep[t][c * nloc[t]:(c + 1) * nloc[t]]
            for s in range(1, MAX_DEPTH + 1):
                counts[t][c, s] = int((sl == s).sum())
        for s in range(1, MAX_DEPTH + 1):
            G[t][s] = _pad_to(int(counts[t][:, s].max()), SBW)

    npad = {}
    for t in ALL_TYPES:
        if t in DST_TYPES:
            rest = max(int(nloc[t] - counts[t][c, 1:].sum()) for c in range(NCORES))
            npad[t] = int(G[t][1:].sum()) + _pad_to(max(rest, 1), SGRP * TILE)
        else:
            npad[t] = nloc[t]

    grp_base = {t: {} for t in DST_TYPES}
    for t in DST_TYPES:
        base = 0
        for s in range(MAX_DEPTH, 0, -1):
            grp_base[t][s] = base
            base += int(G[t][s])
        grp_base[t][0] = base

    perm_pos = {}
    for t in ALL_TYPES:
        pos = np.zeros(NODE_COUNTS[t], np.int64)
        for c in range(NCORES):
            lo, hi = c * nloc[t], (c + 1) * nloc[t]
            if t in DST_TYPES:
                sl = nstep[t][lo:hi]
                p = np.zeros(nloc[t], np.int64)
                for s in list(range(MAX_DEPTH, 0, -1)) + [0]:
                    m = sl == s
                    p[m] = grp_base[t][s] + np.arange(int(m.sum()))
                pos[lo:hi] = p
            else:
                pos[lo:hi] = np.arange(nloc[t])
        perm_pos[t] = pos

    def hw_wrap(q, n):
        """padded position -> wrapped row within a region of n rows."""
        return (q % 128) * (n // 128) + q // 128

    r_type_base = {}
    acc = 0
    for t in R_ORDER:
        r_type_base[t] = acc
        acc += npad[t] * NCORES
    r_rows = acc
    # replica row of global node id (wrapped within its (type, core) region)
    r_row = {}
    for t in ALL_TYPES:
        core = np.arange(NODE_COUNTS[t]) // nloc[t]
        r_row[t] = r_type_base[t] + core * npad[t] + hw_wrap(perm_pos[t], npad[t])

    u_rows, u_off = {}, {}
    for s in range(MAX_DEPTH, 1, -1):
        off, a = {}, 0
        for t in DST_TYPES:
            off[t] = a
            a += int(G[t][s])
        u_off[s], u_rows[s] = off, a

    # ---- per-edge records ------------------------------------------------
    edat = {t: {s: [] for s in range(1, MAX_DEPTH + 1)} for t in DST_TYPES}
    for (st, dt), e in edges.items():
        src, dst = e[0].astype(np.int64), e[1].astype(np.int64)
        s_e = depths[dt][dst].astype(np.int64)
        core = dst // nloc[dt]
        dpos = perm_pos[dt][dst]
        if st in DST_TYPES:
            s_src = nstep[st][src]
            src_core = src // nloc[st]
            row_U = np.zeros_like(src)
            for s2 in range(2, MAX_DEPTH + 1):
                m = s_src == s2
                gq = perm_pos[st][src[m]] - grp_base[st][s2]  # pos within group
                row_U[m] = src_core[m] * u_rows[s2] + u_off[s2][st] + \
                    hw_wrap(gq, int(G[st][s2]))
        else:
            s_src = np.zeros_like(src)
            row_U = np.zeros_like(src)
        fresh = s_src > s_e
        arr_id = np.where(fresh, s_src, 0)
        row = np.where(fresh, row_U, r_row[st][src])
        for s in range(1, MAX_DEPTH + 1):
            m = s_e == s
            edat[dt][s].append(np.stack([core[m], arr_id[m], row[m], dpos[m]], axis=1))

    # ---- static schedule + per-core streams ------------------------------
    plan_calls, tile_maps = {}, {}
    idx_streams = [[] for _ in range(NCORES)]
    dl_streams = [[] for _ in range(NCORES)]
    step_stream_tiles = {}

    for s in range(MAX_DEPTH, 0, -1):
        step_tiles = 0
        for t in DST_TYPES:
            ed = np.concatenate(edat[t][s], axis=0)
            percore, runs = [], {}
            for c in range(NCORES):
                e = ed[ed[:, 0] == c]
                win = (e[:, 2] // WIN).astype(np.int64)
                sb = ((e[:, 3] - grp_base[t][s]) // SBW).astype(np.int64)
                assert (e[:, 3] >= grp_base[t][s]).all()
                assert (e[:, 3] < grp_base[t][s] + G[t][s]).all()
                keys = np.stack([e[:, 1], win, sb], axis=1) if len(e) else np.zeros((0, 3), np.int64)
                for k, v in Counter(map(tuple, keys.tolist())).items():
                    runs[k] = max(runs.get(k, 0), v)
                percore.append((e, keys))
            order = sorted(runs.keys())
            padded = {k: _pad_to(runs[k], TILE) for k in order}
            for k in order:
                assert padded[k] <= MAX_GCALL, f"run too large {k} {padded[k]}"
            calls, cur = [], None
            for k in order:
                a_, w_, sb_ = k
                if cur is not None and cur[0] == (a_, w_) and cur[2] + padded[k] <= MAX_GCALL:
                    cur[1].append((int(sb_), padded[k] // TILE))
                    cur[2] += padded[k]
                else:
                    if cur is not None:
                        calls.append(tuple(cur))
                    cur = [(int(a_), int(w_)), [(int(sb_), padded[k] // TILE)], padded[k]]
            if cur is not None:
                calls.append(tuple(cur))
            plan_calls[(t, s)] = calls
            seq = []
            for ((a_, w_), lst, tot) in calls:
                for (sb_i, nt) in lst:
                    seq.extend([sb_i] * nt)
            first, last = {}, {}
            for i, sb_i in enumerate(seq):
                if sb_i not in first:
                    first[sb_i] = i
                last[sb_i] = i
            tile_maps[(t, s)] = [(sb_i, i == first[sb_i], i == last[sb_i])
                                 for i, sb_i in enumerate(seq)]
            step_tiles += len(seq)
            for c in range(NCORES):
                e, keys = percore[c]
                byrun = defaultdict(list)
                for i in range(len(e)):
                    byrun[tuple(keys[i].tolist())].append(i)
                for ((a_, w_), lst, tot) in calls:
                    for (sb_i, nt) in lst:
                        ii = byrun.get((a_, w_, sb_i), [])
                        rows = e[ii, 2] - w_ * WIN if ii else np.zeros(0, np.int64)
                        dls = e[ii, 3] - grp_base[t][s] - sb_i * SBW if ii else np.zeros(0, np.int64)
                        padn = nt * TILE - len(rows)
                        fill = rows[-1] if len(rows) else 0
                        rows = np.concatenate([rows, np.full(padn, fill, np.int64)])
                        dls = np.concatenate([dls, np.full(padn, -1, np.int64)])
                        assert rows.min(initial=0) >= 0 and rows.max(initial=0) < WIN
                        idx_streams[c].append(rows.astype(np.int16))
                        dl_streams[c].append(dls.astype(np.float32))
        step_stream_tiles[s] = step_tiles

    idx_cat = [np.concatenate(x) for x in idx_streams]
    dl_cat = [np.concatenate(x) for x in dl_streams]
    stream_len = len(idx_cat[0])
    assert all(len(x) == stream_len for x in idx_cat)
    assert stream_len % TILE == 0

    max_nsb = max(int(G[t][s]) // SBW for t in DST_TYPES for s in range(1, MAX_DEPTH + 1))
    assert max_nsb <= 5, f"PSUM budget exceeded: {max_nsb} superblocks"

    return dict(
        nloc=nloc, npad=npad, G=G, grp_base=grp_base, perm_pos=perm_pos,
        r_type_base=r_type_base, r_rows=r_rows,
        u_rows=u_rows, u_off=u_off,
        plan_calls=plan_calls, tile_maps=tile_maps,
        idx_cat=idx_cat, dl_cat=dl_cat,
        ntiles_total=stream_len // TILE, step_stream_tiles=step_stream_tiles,
        nstep=nstep, max_nsb=max_nsb,
    )


# ---------------------------------------------------------------------------
# per-core numpy inputs
# ---------------------------------------------------------------------------

def make_core_inputs(inputs, plan, c):
    nloc, npad = plan["nloc"], plan["npad"]
    perm_pos = plan["perm_pos"]

    xin_cols = []
    for t in ALL_TYPES:
        x = np.asarray(inputs["x_" + t.lower()], np.float32)[c * nloc[t]:(c + 1) * nloc[t]]
        xp = np.zeros((npad[t], INPUT_DIMS[t]), np.float32)
        xp[perm_pos[t][c * nloc[t]:(c + 1) * nloc[t]]] = x
        for k in range(_ceil(INPUT_DIMS[t], 128)):
            blk = np.zeros((128, npad[t]), np.float32)
            w = min(128, INPUT_DIMS[t] - k * 128)
            blk[:w] = xp[:, k * 128:k * 128 + w].T
            xin_cols.append(blk)
    xin = np.concatenate(xin_cols, axis=1)

    params = inputs["params"]
    wcols, bias_cols = [], []

    def add_mat(Wm):
        Wm = np.asarray(Wm, np.float32)
        K, M = Wm.shape
        for k in range(_ceil(K, 128)):
            for m in range(_ceil(M, 128)):
                blk = np.zeros((128, 128), np.float32)
                kk, mm = min(128, K - k * 128), min(128, M - m * 128)
                blk[:kk, :mm] = Wm[k * 128:k * 128 + kk, m * 128:m * 128 + mm]
                wcols.append(blk)

    def add_bias(b):
        b = np.asarray(b, np.float32).reshape(-1)
        for k in range(_ceil(len(b), 128)):
            blk = np.zeros((128, 1), np.float32)
            kk = min(128, len(b) - k * 128)
            blk[:kk, 0] = b[k * 128:k * 128 + kk]
            bias_cols.append(blk)

    for t in ALL_TYPES:
        (W1, b1), (W2, b2) = params["hidden"][t]
        add_mat(W1); add_bias(b1); add_mat(W2); add_bias(b2)
    for t in DST_TYPES:
        (W1, b1), (W2, b2) = params["out"][t]
        add_mat(W1); add_bias(b1); add_mat(W2); add_bias(b2)
    (Wc1, bc1), (Wc2, bc2) = params["classifier"]
    add_mat(Wc1); add_bias(bc1); add_mat(Wc2); add_bias(bc2)
    weights = np.concatenate(wcols, axis=1)
    biases = np.concatenate(bias_cols, axis=1)

    dep = np.asarray(inputs["depth_rel"])[c * nloc["Rel"]:(c + 1) * nloc["Rel"]]
    wpm = np.zeros((npad["Rel"], 128), np.float32)
    gid = np.arange(nloc["Rel"]) // REL_PER_GRAPH
    pos = perm_pos["Rel"][c * nloc["Rel"]:(c + 1) * nloc["Rel"]]
    wpm[pos, gid] = 1.0 / (REL_PER_GRAPH * dep.astype(np.float32))
    nch = npad["Rel"] // 128
    wp_wrapped = wpm.reshape(nch, 128, 128).transpose(1, 0, 2).reshape(128, nch * 128)

    idx = plan["idx_cat"][c]
    idx_w = np.tile(idx.reshape(-1, 16).T, (8, 1))
    dl = plan["dl_cat"][c].reshape(-1, 128).T.copy()

    return {
        "xin": np.ascontiguousarray(xin),
        "weights": np.ascontiguousarray(weights),
        "biases": np.ascontiguousarray(biases),
        "wpool": np.ascontiguousarray(wp_wrapped),
        "gidx": np.ascontiguousarray(idx_w),
        "dstloc": np.ascontiguousarray(dl),
        "iden": np.eye(128, dtype=np.float32),
        "iota": np.tile(np.arange(SBW, dtype=np.float32), (128, 1)),
    }


# ---------------------------------------------------------------------------
# device kernel
# ---------------------------------------------------------------------------

def build_kernel(plan):
    import concourse.bacc as bacc
    import concourse.mybir as mybir
    from concourse import tile

    f32 = mybir.dt.float32
    AF = mybir.ActivationFunctionType
    npad, G, grp_base = plan["npad"], plan["G"], plan["grp_base"]
    u_rows, u_off = plan["u_rows"], plan["u_off"]
    r_rows, r_type_base = plan["r_rows"], plan["r_type_base"]
    max_nsb = plan["max_nsb"]

    nc = bacc.Bacc("TRN2", target_bir_lowering=False, debug=False, num_devices=NCORES)

    xin_cols_n = sum(_ceil(INPUT_DIMS[t], 128) * npad[t] for t in ALL_TYPES)
    xin = nc.dram_tensor("xin", [128, xin_cols_n], f32, kind="ExternalInput")

    nw, wmeta = 0, {}

    def reg_mat(name, K, M):
        nonlocal nw
        wmeta[name] = (nw, _ceil(K, 128), _ceil(M, 128))
        nw += _ceil(K, 128) * _ceil(M, 128)

    nb, bmeta = 0, {}

    def reg_bias(name, L):
        nonlocal nb
        bmeta[name] = (nb, _ceil(L, 128))
        nb += _ceil(L, 128)

    for t in ALL_TYPES:
        reg_mat(("h1", t), INPUT_DIMS[t], HIDDEN_DIM); reg_bias(("h1", t), HIDDEN_DIM)
        reg_mat(("h2", t), HIDDEN_DIM, OUT_DIM); reg_bias(("h2", t), OUT_DIM)
    for t in DST_TYPES:
        reg_mat(("o1", t), 2 * OUT_DIM, HIDDEN_DIM); reg_bias(("o1", t), HIDDEN_DIM)
        reg_mat(("o2", t), HIDDEN_DIM, OUT_DIM); reg_bias(("o2", t), OUT_DIM)
    reg_mat(("c1", None), OUT_DIM, 128); reg_bias(("c1", None), 128)
    reg_mat(("c2", None), 128, 1); reg_bias(("c2", None), 1)

    weights = nc.dram_tensor("weights", [128, nw * 128], f32, kind="ExternalInput")
    biases = nc.dram_tensor("biases", [128, nb], f32, kind="ExternalInput")
    nch_rel = npad["Rel"] // 128
    wpool = nc.dram_tensor("wpool", [128, nch_rel * 128], f32, kind="ExternalInput")
    stream_len = plan["ntiles_total"] * TILE
    gidx = nc.dram_tensor("gidx", [128, stream_len // 16], mybir.dt.int16, kind="ExternalInput")
    dstloc = nc.dram_tensor("dstloc", [128, plan["ntiles_total"]], f32, kind="ExternalInput")
    iden_in = nc.dram_tensor("iden", [128, 128], f32, kind="ExternalInput")
    iota_in = nc.dram_tensor("iota", [128, SBW], f32, kind="ExternalInput")
    y_out = nc.dram_tensor("y", [1, 128], f32, kind="ExternalOutput")

    slab_cols = sum(npad[t] for t in DST_TYPES)
    slab_base, a = {}, 0
    for t in DST_TYPES:
        slab_base[t] = a
        a += npad[t]
    rin_rows = sum(npad[t] for t in ALL_TYPES)
    rin_base, a = {}, 0
    for t in ALL_TYPES:
        rin_base[t] = a
        a += npad[t]
    if DEBUG:
        dbg_hidden = nc.dram_tensor("dbg_hidden", [rin_rows, OUT_DIM], f32, kind="ExternalOutput")
        dbg_final = nc.dram_tensor("dbg_final", [128, slab_cols], f32, kind="ExternalOutput")

    with tile.TileContext(nc) as tc:
        with tc.tile_pool(name="dram", bufs=1, space="DRAM") as dp, \
             tc.tile_pool(name="persist", bufs=1) as pp:

            R = dp.tile([r_rows, OUT_DIM], f32, tag="R")
            R_in = dp.tile([rin_rows, OUT_DIM], f32, tag="R_in")
            U_in = {s: dp.tile([u_rows[s], OUT_DIM], f32, tag=f"U_in{s}", name=f"U_in{s}") for s in u_rows}
            U_full = {s: dp.tile([NCORES * u_rows[s], OUT_DIM], f32, tag=f"U_full{s}", name=f"U_full{s}")
                      for s in u_rows}

            xs = pp.tile([128, slab_cols], f32)
            wsb = pp.tile([128, nw * 128], f32)
            bsb = pp.tile([128, nb], f32)
            iden = pp.tile([128, 128], f32)
            iota = pp.tile([128, SBW], f32)
            dl_sb = pp.tile([128, plan["ntiles_total"]], f32)

            nc.sync.dma_start(wsb[:], weights[:])
            nc.sync.dma_start(bsb[:], biases[:])
            nc.sync.dma_start(iden[:], iden_in[:])
            nc.sync.dma_start(iota[:], iota_in[:])
            nc.sync.dma_start(dl_sb[:], dstloc[:])

            def W(name, k, m):
                base, nk, nm = wmeta[name]
                j = base + k * nm + m
                return wsb[:, j * 128:(j + 1) * 128]

            def Bv(name, k):
                base, _ = bmeta[name]
                return bsb[:, base + k:base + k + 1]

            def make_mlp2(wpool_, pspool_):
                def mlp2(dst_ap, rhs_list, w1, w2, n):
                    assert n <= NCHUNK
                    h = wpool_.tile([128, 2, NCHUNK], f32, tag="mlph")
                    ps = pspool_.tile([128, 2, NCHUNK], f32, tag="mlpps1")
                    for m in range(2):
                        for ki, rhs in enumerate(rhs_list):
                            nc.tensor.matmul(ps[:, m, :n], W(w1, ki, m), rhs,
                                             start=(ki == 0), stop=(ki == len(rhs_list) - 1))
                        nc.scalar.activation(h[:, m, :n], ps[:, m, :n], AF.Gelu, bias=Bv(w1, m))
                    ps2 = pspool_.tile([128, NCHUNK], f32, tag="mlpps2")
                    for ki in range(2):
                        nc.tensor.matmul(ps2[:, :n], W(w2, ki, 0), h[:, ki, :n],
                                         start=(ki == 0), stop=(ki == 1))
                    nc.scalar.activation(dst_ap, ps2[:, :n], AF.Identity, bias=Bv(w2, 0))
                return mlp2

            def make_stager(wpool_, pspool_):
                def stage(src_fm_ap, dram_region_ap, region_rows, q0, ncols):
                    """Write feature-major [128, ncols] (padded positions
                    q0..q0+ncols) into wrapped node-major DRAM region."""
                    nrow = region_rows // 128
                    wrapped = dram_region_ap.rearrange("(p j) f -> p (j f)", p=128)
                    for g0 in range(0, ncols, SGRP * 128):
                        gn = min(SGRP * 128, ncols - g0)
                        nmt = wpool_.tile([128, SGRP, 128], f32, tag="nmt", bufs=2)
                        for j0 in range(0, gn, 128):
                            pst = pspool_.tile([128, 128], f32, tag="tpst")
                            nc.tensor.transpose(pst[:], src_fm_ap[:, g0 + j0:g0 + j0 + 128], iden[:])
                            nc.vector.tensor_copy(nmt[:, j0 // 128, :], pst[:])
                        j = (q0 + g0) // 128
                        nc.sync.dma_start(
                            wrapped[:, j * OUT_DIM:(j + gn // 128) * OUT_DIM],
                            nmt[:, :gn // 128, :])
                return stage

            # ================= hidden MLP phase ===========================
            xin_off = {}
            o = 0
            for t in ALL_TYPES:
                xin_off[t] = o
                o += _ceil(INPUT_DIMS[t], 128) * npad[t]

            with tc.tile_pool(name="hidw", bufs=3) as hw, \
                 tc.tile_pool(name="hidps", bufs=1, space="PSUM") as hps:
                mlp2 = make_mlp2(hw, hps)
                stage = make_stager(hw, hps)
                for t in ALL_TYPES:
                    nchk = _ceil(INPUT_DIMS[t], 128)
                    is_dst = t in DST_TYPES
                    for g0 in range(0, npad[t], SGRP * 128):
                        gn = min(SGRP * 128, npad[t] - g0)
                        if is_dst:
                            out_fm = xs[:, slab_base[t] + g0: slab_base[t] + g0 + gn]
                        else:
                            tmp = hw.tile([128, SGRP * 128], f32, tag="hidtmp")
                            out_fm = tmp[:, :gn]
                        for j0 in range(0, gn, NCHUNK):
                            n = min(NCHUNK, gn - j0)
                            rhs_list = []
                            for k in range(nchk):
                                xc = hw.tile([128, NCHUNK], f32, tag="xinc")
                                nc.sync.dma_start(
                                    xc[:, :n],
                                    xin[:, xin_off[t] + k * npad[t] + g0 + j0:
                                        xin_off[t] + k * npad[t] + g0 + j0 + n])
                                rhs_list.append(xc[:, :n])
                            mlp2(out_fm[:, j0:j0 + n], rhs_list, ("h1", t), ("h2", t), n)
                        stage(out_fm, R_in[rin_base[t]: rin_base[t] + npad[t], :],
                              npad[t], g0, gn)

            if DEBUG:
                nc.sync.dma_start(dbg_hidden[:, :], R_in[:, :])

            # ================= replica build ==============================
            for t in ALL_TYPES:
                nc.gpsimd.collective_compute(
                    "AllGather", mybir.AluOpType.bypass,
                    replica_groups=[list(range(NCORES))],
                    ins=[R_in[rin_base[t]: rin_base[t] + npad[t], :].opt()],
                    outs=[R[r_type_base[t]: r_type_base[t] + NCORES * npad[t], :].opt()],
                )

            # ================= message passing ============================
            with tc.tile_pool(name="msgs", bufs=2) as sp, \
                 tc.tile_pool(name="msgg", bufs=2) as gp, \
                 tc.tile_pool(name="msgw", bufs=3) as mw, \
                 tc.tile_pool(name="msgb", bufs=1) as mb, \
                 tc.tile_pool(name="msgps", bufs=1, space="PSUM") as mps, \
                 tc.tile_pool(name="msgagg", bufs=1, space="PSUM") as aggpool:
                mlp2 = make_mlp2(mw, mps)
                stage = make_stager(mw, mps)
                idx_pos = 0
                tile_pos = 0
                for s in range(MAX_DEPTH, 0, -1):
                    ntile_s = plan["step_stream_tiles"][s]
                    gs = sp.tile([128, ntile_s * TILE // 16], mybir.dt.int16, tag="gs")
                    nc.sync.dma_start(
                        gs[:], gidx[:, idx_pos // 16:(idx_pos + ntile_s * TILE) // 16])
                    step_idx0 = idx_pos
                    for t in DST_TYPES:
                        nsb = int(G[t][s]) // SBW
                        if nsb == 0:
                            continue
                        tmap = plan["tile_maps"][(t, s)]
                        touched = set(sb for (sb, _f, _l) in tmap)
                        agg = aggpool.tile([128, max_nsb, SBW], f32, tag="agg")
                        lt = 0
                        for ((arr, w), lst, tot) in plan["plan_calls"][(t, s)]:
                            gt = gp.tile([128, MAX_GCALL // 128, 128], f32, tag="gt")
                            ncols = tot // 128
                            if arr == 0:
                                src_ap = R[w * WIN: min((w + 1) * WIN, r_rows), :]
                            else:
                                src_ap = U_full[arr][w * WIN: min((w + 1) * WIN, NCORES * u_rows[arr]), :]
                            oo = idx_pos - step_idx0
                            nc.gpsimd.dma_gather(
                                gt[:, :ncols, :], src_ap,
                                gs[:, oo // 16:(oo + tot) // 16],
                                tot, tot, OUT_DIM, single_packet=False)
                            idx_pos += tot
                            col = 0
                            for (sb_i, nt) in lst:
                                for _j in range(nt):
                                    sb_t, first, last_f = tmap[lt]
                                    oh = mw.tile([128, SBW], f32, tag="oh", bufs=2)
                                    nc.vector.tensor_scalar(
                                        oh[:], iota[:], dl_sb[:, tile_pos:tile_pos + 1],
                                        None, op0=mybir.AluOpType.is_equal)
                                    nc.tensor.matmul(agg[:, sb_i, :], gt[:, col, :], oh[:],
                                                     start=first, stop=last_f,
                                                     skip_group_check=True)
                                    lt += 1
                                    tile_pos += 1
                                    col += 1
                        aggs = mb.tile([128, max_nsb * SBW], f32, tag="aggs")
                        for sb_i in range(nsb):
                            if sb_i in touched:
                                nc.vector.tensor_copy(
                                    aggs[:, sb_i * SBW:(sb_i + 1) * SBW], agg[:, sb_i, :])
                            else:
                                nc.vector.memset(aggs[:, sb_i * SBW:(sb_i + 1) * SBW], 0.0)
                        gbase = grp_base[t][s]
                        upd = aggs
                        for j0 in range(0, nsb * SBW, NCHUNK):
                            n = min(NCHUNK, nsb * SBW - j0)
                            mlp2(upd[:, j0:j0 + n],
                                 [aggs[:, j0:j0 + n],
                                  xs[:, slab_base[t] + gbase + j0: slab_base[t] + gbase + j0 + n]],
                                 ("o1", t), ("o2", t), n)
                        nc.vector.tensor_copy(
                            xs[:, slab_base[t] + gbase: slab_base[t] + gbase + nsb * SBW],
                            upd[:, :nsb * SBW])
                        if s >= 2:
                            stage(upd[:, :nsb * SBW],
                                  U_in[s][u_off[s][t]: u_off[s][t] + int(G[t][s]), :],
                                  int(G[t][s]), 0, nsb * SBW)
                    if s >= 2:
                        nc.gpsimd.collective_compute(
                            "AllGather", mybir.AluOpType.bypass,
                            replica_groups=[list(range(NCORES))],
                            ins=[U_in[s][:, :].opt()],
                            outs=[U_full[s][:, :].opt()],
                        )

            if DEBUG:
                nc.sync.dma_start(dbg_final[:, :], xs[:, :])

            # ================= pooling + classifier =======================
            with tc.tile_pool(name="poolw", bufs=2) as pw, \
                 tc.tile_pool(name="poolps", bufs=1, space="PSUM") as pps:
                wp_sb = pw.tile([128, nch_rel * 128], f32, tag="wpool")
                nc.sync.dma_start(wp_sb[:], wpool[:])
                pool_ps = pps.tile([128, 128], f32, tag="poolps")
                for ch in range(nch_rel):
                    pst = pps.tile([128, 128], f32, tag="tpst")
                    relT = pw.tile([128, 128], f32, tag="relT")
                    nc.tensor.transpose(
                        pst[:], xs[:, slab_base["Rel"] + ch * 128: slab_base["Rel"] + (ch + 1) * 128],
                        iden[:])
                    nc.vector.tensor_copy(relT[:], pst[:])
                    nc.tensor.matmul(pool_ps[:], wp_sb[:, ch * 128:(ch + 1) * 128], relT[:],
                                     start=(ch == 0), stop=(ch == nch_rel - 1))
                pooled_nm = pw.tile([128, 128], f32, tag="pooled")
                nc.vector.tensor_copy(pooled_nm[:], pool_ps[:])
                pst = pps.tile([128, 128], f32, tag="tpst")
                pooled_fm = pw.tile([128, 128], f32, tag="pooledfm")
                nc.tensor.transpose(pst[:], pooled_nm[:], iden[:])
                nc.vector.tensor_copy(pooled_fm[:], pst[:])
                hc_ps = pps.tile([128, 128], f32, tag="poolps")
                nc.tensor.matmul(hc_ps[:], W(("c1", None), 0, 0), pooled_fm[:], start=True, stop=True)
                hc = pw.tile([128, 128], f32, tag="hc")
                nc.scalar.activation(hc[:], hc_ps[:], AF.Gelu, bias=Bv(("c1", None), 0))
                y_ps = pps.tile([1, 128], f32, tag="tpst")
                nc.tensor.matmul(y_ps[:], W(("c2", None), 0, 0)[:, :1], hc[:], start=True, stop=True)
                y_sb = pw.tile([1, 128], f32, tag="ysb")
                nc.scalar.activation(y_sb[:], y_ps[:], AF.Identity, bias=Bv(("c2", None), 0)[:1, :])
                nc.sync.dma_start(y_out[:], y_sb[:])

    return nc


# ---------------------------------------------------------------------------
# entry point
# ---------------------------------------------------------------------------

def kernel(**inputs):
    from concourse.bass_utils import run_bass_kernel_spmd

    plan = make_plan(inputs)
    nc = build_kernel(plan)
    nc.compile()
    in_maps = [make_core_inputs(inputs, plan, c) for c in range(NCORES)]
    res = run_bass_kernel_spmd(nc, in_maps, core_ids=list(range(NCORES)))
    y = np.concatenate([np.asarray(res.results[c]["y"]).reshape(128) for c in range(NCORES)])
    if DEBUG:
        kernel._last = (plan, res)
    return y.reshape(1024, 1).astype(np.float32)
